# revision 30
# baseline (speedup 1.0000x reference)
"""BTT layer kernel for Trainium2 (8 NeuronCores, data-parallel over batch).

Computes y = BTT(x; W1, W2) where
  x: (4096, 4096) fp32, W1: (64, 64, 256) fp32, W2: (64, 256, 64) fp32
  stage 1: t[b, m2, n1, r] = sum_m1 x[b, m2, m1] * W1[m2, m1, n1*4+r]
  stage 2: y[b, n1, n2]   = sum_{m2, r} t[b, m2, n1, r] * W2[n1, m2*4+r, n2]

Design (SBUF-resident intermediate, no DRAM round-trip; ~128us HW,
rel err 5.5e-4; the previous DRAM-round-trip version measured ~190us):
  - everything fp16 on device (x, W1, W2, t, y); host casts / uncasts.
  - stage 1: per m2-pair: 2-way row-packed K=64 matmuls (tile_position rows
    (0,0)/(64,0)) fill a 2-bank PSUM tile (concurrent row-group matmuls
    must land in DIFFERENT PSUM banks); ACT/DVE copies (cast to fp16)
    drain into t1[j] tiles laid out [128 p=32r+n1p][h][m2p][b].  Drains
    are the throughput wall: fp32 PSUM sources run at 1 elem/cycle/lane
    on both DVE (0.96 GHz) and ACT (1.2 GHz); GPSIMD has no PSUM port.
  - the BTT "transpose" (n1p <-> m2p inside 32-lane banks; r rides as the
    bank index) runs on the DVE stream-transpose as fp32 *pairs* of
    b-adjacent fp16 values, landing t2 quads [128 k=32r+m2p][n1p][b]
    directly in stage-2 moving-operand layout.  The reshape front-end
    moves 16 bits/lane/cycle, so this is ~8us per 2.1MB quad, ~64us
    total -- the DVE is the kernel's critical resource.  (DMA-routed
    alternatives through DRAM measured strictly worse: 512B-granule
    scatter reads OR writes collapse SDMA efficiency.)
  - stage 2: per n1-pair (32h+q16, 32h+q16+16): 2-way column-packed K=128
    matmuls (tile_position cols (0,0)/(0,64), same bank is fine for
    column packing), j=0/1 accumulate in 1-bank PSUM groups of 2 n1;
    drained with cast to fp16 and DMA'd out.  Pairing within an h-half
    lets stage-2 consume t2 per-h, so 4 rotating quad buffers suffice.
  - batch chunked (BC=256) x 2; emission interleaves st2(c-1) h-blocks
    into st1(c) so ACT drains, DVE transposes, and PE matmuls of
    adjacent chunks overlap.  DVE st1-drain shares live only in the j0
    half (j1 shares would queue behind transposes in the DVE FIFO and
    stall the ps1 rotation); stage-2 drains stay on ACT.
  - startup: the first x quarter and W1 piece go FIRST on the sync
    HWDGE ring so the SDMA packet round-robin cannot dilute them with
    later loads (all early loads on one FIFO ring in need-order; W2 on
    gpsimd after stage-1 emission begins).  Going finer than 0.5MB
    pieces measured worse (per-DMA issue+completion overhead).
  - tail: the last chunk's h1 stage-2 drains alternate ACT/DVE (both
    engines are idle then), collapsing the stage-2 MM->drain ping-pong;
    applying the same split mid-kernel measured worse (the DVE share
    queues behind transposes and delays them).  ysb bufs=6 decouples
    the drain chain from y-store WAR.
  - the early emission of st2(c-1) h-blocks inside st1(c) is
    load-bearing: it frees the rotating t2 quad buffers before chunk
    c's transposes need them.  Deferring st2(c-1) to chain the drain
    phases tighter measured 139us (transposes block on the quad WAR;
    no SBUF room for more quad buffers).

Self-contained: hardcodes all shapes; imports the Bass toolchain from
/opt/trn_rl_repo.
"""

import os
import re
import sys

import numpy as np

sys.path.insert(0, "/opt/trn_rl_repo")

import bass_rust  # noqa: E402
import concourse.bass as bass  # noqa: E402
import concourse.mybir as mybir  # noqa: E402
import concourse.tile as tile  # noqa: E402
from concourse import bass_utils  # noqa: E402

# ----------------------------------------------------------------------------
# Environment shims (same as baseline)
# ----------------------------------------------------------------------------


def _install_walrus_single_wait_patch():
    """This container's walrus build supports only ONE sem-wait per
    instruction. TileContext attaches several. Split every multi-wait
    instruction: hoist all-but-one wait onto same-engine NoOps placed
    immediately before it, and emit the tail drain one proc at a time."""
    if getattr(tile.TileContext, "_single_wait_patched", False):
        return

    counter = [0]

    def _split_multiwait_insts(ordered):
        for insts in ordered.values():
            i = 0
            while i < len(insts):
                inst = insts[i]
                si = getattr(inst, "sync_info", None)
                if si is not None and len(si.on_wait) > 1:
                    waits = list(si.on_wait)
                    new_nops = []
                    for w in waits[:-1]:
                        counter[0] += 1
                        nop = mybir.InstNoOp(
                            name=f"waitsplit_{counter[0]}", ins=[], outs=[]
                        )
                        nop.engine = inst.engine
                        nop.sync_info = bass_rust.SyncInfo(on_wait=[w], on_update=[])
                        new_nops.append(nop)
                    inst.sync_info = bass_rust.SyncInfo(
                        on_wait=[waits[-1]], on_update=list(si.on_update)
                    )
                    insts[i:i] = new_nops
                    i += len(new_nops)
                i += 1

    orig_lower = tile.TileContext._lower_ordered_insts

    def patched_lower(self, ordered):
        _split_multiwait_insts(ordered)
        return orig_lower(self, ordered)

    def split_drain_and_barrier(self, tick_clock, wait_clock):
        gc = tick_clock.global_clock
        ticks = [int(x) for x in re.findall(r"\d+", repr(gc.copy()))]
        emitted = False
        for i, t in enumerate(ticks):
            if t > 0:
                vec = [0] * len(ticks)
                vec[i] = t
                drain_inst = self.nc.sync.drain()
                wait_clock.add_sem_waits(
                    drain_inst.ins,
                    bass_rust.ScopedClock({None: bass_rust.VectorClock(vec)}),
                )
                emitted = True
        if not emitted:
            self.nc.sync.drain()
        self.nc.all_engine_barrier()
        assert self.sems is not None
        popped = self.nc._tile_sem_poison_stack.pop()
        assert popped is self._sem_poison
        self.nc.clear_and_free_semaphores(list(self.sems.allocated().values()))
        self.nc.all_engine_barrier()

    tile.TileContext._lower_ordered_insts = patched_lower
    tile.TileContext._drain_and_barrier = split_drain_and_barrier
    tile.TileContext._single_wait_patched = True


def _install_ntff_hook():
    """Register the NTFF profiling hook (missing antenv.axon_hooks module in
    this image). Only needed when profiling; harmless otherwise."""
    import types

    if "antenv.axon_hooks" not in sys.modules:
        import antenv

        mod = types.ModuleType("antenv.axon_hooks")
        mod._hook = None
        mod.set_axon_ntff_profile_hook = lambda h: setattr(mod, "_hook", h)
        mod.get_axon_ntff_profile_hook = lambda: mod._hook
        sys.modules["antenv.axon_hooks"] = mod
        antenv.axon_hooks = mod
    m = sys.modules["antenv.axon_hooks"]
    if m._hook is None:
        try:
            from trn_agent_boot.trn_boot import _ntff_profile_via_ctypes

            m.set_axon_ntff_profile_hook(
                _ntff_profile_via_ctypes("/opt/axon/libaxon_pjrt.so")
            )
        except Exception:
            pass
    bass_utils.upload_artifacts = lambda d: d


_install_walrus_single_wait_patch()

# ----------------------------------------------------------------------------
# Problem constants / tunables
# ----------------------------------------------------------------------------

B = 4096
M1 = M2 = N1 = N2 = 64
R = 4
NCORES = 8
BP = B // NCORES  # batch rows per core (512)

BC = int(os.environ.get("BTT_BC", "256"))  # batch chunk
# every Nth stage-1/stage-2 drain goes to DVE instead of ACT (0 = all ACT,
# 1 = all DVE)
S1_DVE_MOD = int(os.environ.get("BTT_S1_DVE_MOD", "10"))
S2_DVE_MOD = int(os.environ.get("BTT_S2_DVE_MOD", "1"))
# how many of the 4 per-chunk transpose quads go via DMA DRAM round-trip
# instead of the DVE stream-transpose (0..4, j0 quads first)
DMA_QUADS = int(os.environ.get("BTT_DMA_QUADS", "2"))


# ----------------------------------------------------------------------------
# Bass program
# ----------------------------------------------------------------------------


def build_program(bc=None, s1_dve_mod=None, s2_dve_mod=None, dma_quads=None):
    bc = bc or BC
    s1m = S1_DVE_MOD if s1_dve_mod is None else s1_dve_mod
    s2m = S2_DVE_MOD if s2_dve_mod is None else s2_dve_mod
    dmaq = DMA_QUADS if dma_quads is None else dma_quads
    nch = BP // bc
    f16 = mybir.dt.float16
    f32 = mybir.dt.float32
    sub = bc // 2  # matmul moving-column sub-chunk

    nc = bass.Bass(
        "TRN2",
        target_bir_lowering=False,
        debug=False,
        detect_race_conditions=os.environ.get("BTT_NO_RACE", "0") != "1",
    )

    # Host-marshalled layouts (see _marshal_inputs):
    #   xt[p][c][g][b] = x[core*BP + c*bc + b, (2g + p//64)*64 + p%64]
    #       (p = m1 + 64*(m2%2), g = m2//2)
    #   w1[p][g][c2]   = W1[2g + p//64, p%64, n1*4+r] with c2 = h*128+r*32+n1p,
    #       n1 = 32h + n1p
    #   w2[k][n1][j][n2] = W2[n1, (32j + k%32)*4 + k//32, n2]   (k = 32r + m2p)
    #   yt[p][c][qy][b] = y[core*BP + c*bc + b, n1*64 + n2]
    #       (n1 = 32*(qy//16) + qy%16 + 16*(p//64), n2 = p%64)
    xt_d = nc.dram_tensor("xt", [128, nch, 32, bc], f16, kind="ExternalInput")
    w1_d = nc.dram_tensor("w1", [128, 32, 256], f16, kind="ExternalInput")
    w2_d = nc.dram_tensor("w2", [128, 64, 2, 64], f16, kind="ExternalInput")
    yt_d = nc.dram_tensor("yt", [128, nch, 32, bc], f16, kind="ExternalOutput")

    with tile.TileContext(nc) as tc:
        with (
            tc.tile_pool(name="weights", bufs=1) as wpool,
            tc.tile_pool(name="xin", bufs=2) as xpool,
            tc.tile_pool(name="t1", bufs=1) as t1pool,
            tc.tile_pool(name="t2", bufs=4) as t2pool,
            tc.tile_pool(name="yout", bufs=6) as ypool,
            tc.tile_pool(name="ps1", bufs=3, space="PSUM") as ps1pool,
            tc.tile_pool(name="ps2", bufs=2, space="PSUM") as ps2pool,
            tc.tile_pool(name="dram", bufs=1, space="DRAM") as dram_pool,
        ):
            w1_sb = wpool.tile([128, 32, 256], f16, name="w1_sb")
            # DRAM scratch for DMA-routed transpose quads, slot per (c%2, j, h)
            qd_d = dram_pool.tile([2, 2, 2, 128, 32, bc], f16, name="qscratch")
            w2_sb = wpool.tile([128, 64, 2, 64], f16, name="w2_sb")

            # t1[j]: [128 p=32r+n1p][h][m2p][b]  (stage-1 output layout)
            t1_sb = [
                t1pool.tile([128, 2, 32, bc], f16, name=f"t1_{j}") for j in range(2)
            ]
            # t2 quads: [128 k=32r+m2p][n1p][b], rotated via tag (bufs=4)
            t2_sb = {}

            xg = [None] * nch

            def load_x(c, half=None, quarter=None):
                if quarter == 0 or half == 0 or (half is None and quarter is None):
                    xg[c] = xpool.tile([128, 32, bc], f16, tag="xg", name=f"xg_{c}")
                if half is None and quarter is None:
                    nc.sync.dma_start(xg[c][:], xt_d[:, c, :, :])
                elif quarter is not None:
                    nc.sync.dma_start(
                        xg[c][:, 8 * quarter : 8 * quarter + 8, :],
                        xt_d[:, c, 8 * quarter : 8 * quarter + 8, :],
                    )
                else:
                    nc.sync.dma_start(
                        xg[c][:, 16 * half : 16 * half + 16, :],
                        xt_d[:, c, 16 * half : 16 * half + 16, :],
                    )

            def load_w1(k, eng=None):
                (eng or nc.scalar).dma_start(
                    w1_sb[:, 8 * k : 8 * k + 8, :], w1_d[:, 8 * k : 8 * k + 8, :]
                )

            def load_w2():
                nc.gpsimd.dma_start(w2_sb[:, 0:32, :, :], w2_d[:, 0:32, :, :])
                nc.gpsimd.dma_start(w2_sb[:, 32:64, :, :], w2_d[:, 32:64, :, :])

            def stage1_half(c, jhalf):
                # g indexes an m2-pair (2g, 2g+1); j = m2//32 = g//16
                for g in range(16 * jhalf, 16 * jhalf + 16):
                    j = jhalf
                    mp = (2 * g) % 32
                    # p (row-group) must select the PSUM bank: concurrent
                    # row-packed matmuls cannot share a bank
                    ps = ps1pool.tile([128, 2, 2, sub * 2], f32, tag="ps1", name=f"ps1_{c}_{g}")
                    for h in range(2):
                        for p in range(2):
                            for s in range(2):
                                nc.tensor.matmul(
                                    ps[:, p, h, s * sub : (s + 1) * sub],
                                    w1_sb[
                                        64 * p : 64 * p + 64,
                                        g,
                                        128 * h : 128 * h + 128,
                                    ],
                                    xg[c][
                                        64 * p : 64 * p + 64,
                                        g,
                                        s * sub : (s + 1) * sub,
                                    ],
                                    start=True,
                                    stop=True,
                                )
                    # one (128, 1024) drain per g: PSUM fp32 -> t1 fp16
                    dst = t1_sb[j][:, :, mp : mp + 2, :]
                    src = ps.rearrange("c p h b -> c h p b")
                    if s1m > 0 and g % s1m == s1m - 1:
                        nc.vector.tensor_copy(dst, src)
                    else:
                        nc.scalar.copy(dst, src)
                    if c + 1 < nch and g % 16 == 1:
                        load_x(c + 1, jhalf)
                    if c == 0 and jhalf == 0 and g in (1, 3, 5):
                        load_w1((g + 1) // 2, eng=nc.sync)
                    if c == 0 and jhalf == 0 and g in (1, 2, 3):
                        load_x(0, quarter=g)

            def transpose(j, h, c):
                #   t2q[32r+m2p][n1p][b] = t1[j][32r+n1p][h][m2p][b]
                t2q = t2pool.tile([128, 32, bc], f16, tag="t2q", name=f"t2q_{j}_{h}")
                t2_sb[(j, h)] = t2q
                n_dma = [(0, 0), (0, 1), (1, 0), (1, 1)][:dmaq]
                if (j, h) in n_dma:
                    # DMA round-trip through DRAM with the shuffle applied on
                    # the WRITE side (posted writes tolerate 512B runs), then
                    # a fast contiguous read-back.  qd is in t2 layout
                    # [k=32r+m2p][n1p][b].
                    qd = qd_d[c % 2, j, h]
                    for r in range(R):
                        nc.scalar.dma_start(
                            qd[32 * r : 32 * r + 32].rearrange("m n b -> n m b"),
                            t1_sb[j][32 * r : 32 * r + 32, h],
                        )
                    nc.gpsimd.dma_start(t2q[:], qd[:])
                else:
                    # DVE 32x32 stream-transpose (fp32 pairs), split along b
                    # for finer pipelining with stage-2
                    b2 = bc // 4  # fp32 units per half
                    for v in range(2):
                        in_ = (
                            t1_sb[j][:, h]
                            .bitcast(f32)[:, :, v * b2 : (v + 1) * b2]
                            .rearrange("p m b -> p b m")
                        )
                        out = (
                            t2q.bitcast(f32)[:, :, v * b2 : (v + 1) * b2]
                            .rearrange("p n b -> p b n")
                        )
                        nc.vector.transpose(out, in_)

            def stage2_half(c, h):
                # n1-pair (32h + q16, 32h + q16 + 16); consumes quads (*, h)
                for qg in range(8):
                    ps = ps2pool.tile([128, 2, bc], f32, tag="ps2", name=f"ps2_{c}_{h}_{qg}")
                    hb = bc // 2
                    for u in range(2):
                        q16 = 2 * qg + u
                        for v in range(2):
                            for j in range(2):
                                for pp in range(2):
                                    nc.tensor.matmul(
                                        ps[
                                            64 * pp : 64 * pp + 64,
                                            u,
                                            v * hb : (v + 1) * hb,
                                        ],
                                        w2_sb[:, 32 * h + q16 + 16 * pp, j, :],
                                        t2_sb[(j, h)][
                                            :, q16 + 16 * pp, v * hb : (v + 1) * hb
                                        ],
                                        start=(j == 0),
                                        stop=(j == 1),
                                    )
                    ysb = ypool.tile([128, 2, bc], f16, tag="ysb", name=f"ysb_{c}_{h}_{qg}")
                    use_dve = (s2m > 0 and qg % s2m == s2m - 1) or (
                        c == nch - 1 and h == 1 and qg % 2 == 1
                    )
                    if use_dve:
                        nc.vector.tensor_copy(ysb[:], ps[:])
                    else:
                        nc.scalar.copy(ysb[:], ps[:])
                    nc.sync.dma_start(
                        yt_d[:, c, 16 * h + 2 * qg : 16 * h + 2 * qg + 2, :], ysb[:]
                    )

            # ---- pipelined emission across chunks -------------------------
            load_x(0, quarter=0)
            load_w1(0, eng=nc.sync)
            stage1_half(0, 0)
            load_w2()
            transpose(0, 0, 0)
            transpose(0, 1, 0)
            stage1_half(0, 1)
            transpose(1, 0, 0)
            transpose(1, 1, 0)
            for c in range(1, nch):
                stage2_half(c - 1, 0)
                stage1_half(c, 0)
                stage2_half(c - 1, 1)
                transpose(0, 0, c)
                transpose(0, 1, c)
                stage1_half(c, 1)
                transpose(1, 0, c)
                transpose(1, 1, c)
            stage2_half(nch - 1, 0)
            stage2_half(nch - 1, 1)

    return nc


# ----------------------------------------------------------------------------
# Host marshalling
# ----------------------------------------------------------------------------


def _marshal_inputs(x, W1, W2, bc):
    nch = BP // bc
    # x: (B, 4096) -> xt_all (128, 32, B) with p = m1 + 64*(m2%2), g = m2//2
    xr = x.reshape(B, 32, 2, 64)  # [b][g][par][m1]
    xt_all = np.ascontiguousarray(
        xr.transpose(2, 3, 1, 0).reshape(128, 32, B).astype(np.float16)
    )
    # W1 (64 m2, 64 m1, 256 c=n1*4+r) -> w1[p][g][c2], c2 = h*128 + r*32 + n1p
    w1r = W1.reshape(32, 2, 64, 2, 32, 4)  # [g][par][m1][h][n1p][r]
    w1 = np.ascontiguousarray(
        w1r.transpose(1, 2, 0, 3, 5, 4).reshape(128, 32, 256).astype(np.float16)
    )
    # W2 (64 n1, 256 d=(32j+m2p)*4+r, 64 n2) -> w2[k=32r+m2p][n1][j][n2]
    w2r = W2.reshape(64, 2, 32, 4, 64)  # [n1][j][m2p][r][n2]
    w2 = np.ascontiguousarray(
        w2r.transpose(3, 2, 0, 1, 4).reshape(128, 64, 2, 64).astype(np.float16)
    )

    in_maps = []
    for core in range(NCORES):
        xc = xt_all[:, :, core * BP : (core + 1) * BP]  # (128, 32, BP)
        xc = np.ascontiguousarray(
            xc.reshape(128, 32, nch, bc).transpose(0, 2, 1, 3)
        )  # [p][c][g][b]
        in_maps.append({"xt": xc, "w1": w1, "w2": w2})
    return in_maps


def _unmarshal_output(results, bc):
    nch = BP // bc
    y = np.empty((B, N1 * N2), np.float32)
    for core, res in enumerate(results):
        yt = res["yt"]  # (128, nch, 32, bc) fp16
        # p = pp*64 + n2; qy = h*16 + q16; n1 = 32h + q16 + 16pp
        yc = (
            yt.reshape(2, 64, nch, 2, 16, bc)  # [pp][n2][c][h][q16][b]
            .transpose(2, 5, 3, 0, 4, 1)  # [c][b][h][pp][q16][n2]
            .reshape(BP, 4096)
            .astype(np.float32)
        )
        y[core * BP : (core + 1) * BP] = yc
    return y


# ----------------------------------------------------------------------------
# Public entry point
# ----------------------------------------------------------------------------

_PROGRAM_CACHE = {}


def kernel(x, W1, W2, _trace=False, _config=None):
    cfg = _config or {}
    key = tuple(sorted(cfg.items())) if cfg else None
    if key not in _PROGRAM_CACHE:
        _PROGRAM_CACHE[key] = build_program(**cfg)
    nc = _PROGRAM_CACHE[key]

    bc = cfg.get("bc", BC)
    in_maps = _marshal_inputs(
        np.asarray(x, np.float32),
        np.asarray(W1, np.float32),
        np.asarray(W2, np.float32),
        bc,
    )
    if _trace:
        _install_ntff_hook()
        os.environ["BASS_PERFETTO_PROFILE_ALL_CORES"] = "1"
    res = bass_utils.run_bass_kernel_spmd(
        nc, in_maps, core_ids=list(range(NCORES)), trace=_trace
    )
    y = _unmarshal_output(res.results, bc)
    if _trace:
        return y, res
    return y


# revision 31
# speedup vs baseline: 1.1737x; 1.1737x over previous
"""BTT layer kernel for Trainium2 (8 NeuronCores, data-parallel over batch).

Computes y = BTT(x; W1, W2) where
  x: (4096, 4096) fp32, W1: (64, 64, 256) fp32, W2: (64, 256, 64) fp32
  stage 1: t[b, m2, n1, r] = sum_m1 x[b, m2, m1] * W1[m2, m1, n1*4+r]
  stage 2: y[b, n1, n2]   = sum_{m2, r} t[b, m2, n1, r] * W2[n1, m2*4+r, n2]

Design (SBUF-resident intermediate, no DRAM round-trip; ~128us HW,
rel err 5.5e-4; the previous DRAM-round-trip version measured ~190us):
  - everything fp16 on device (x, W1, W2, t, y); host casts / uncasts.
  - stage 1: per m2-pair: 2-way row-packed K=64 matmuls (tile_position rows
    (0,0)/(64,0)) fill a 2-bank PSUM tile (concurrent row-group matmuls
    must land in DIFFERENT PSUM banks); ACT/DVE copies (cast to fp16)
    drain into t1[j] tiles laid out [128 p=32r+n1p][h][m2p][b].  Drains
    are the throughput wall: fp32 PSUM sources run at 1 elem/cycle/lane
    on both DVE (0.96 GHz) and ACT (1.2 GHz); GPSIMD has no PSUM port.
  - the BTT "transpose" (n1p <-> m2p inside 32-lane banks; r rides as the
    bank index) runs on the DVE stream-transpose as fp32 *pairs* of
    b-adjacent fp16 values, landing t2 quads [128 k=32r+m2p][n1p][b]
    directly in stage-2 moving-operand layout.  The reshape front-end
    moves 16 bits/lane/cycle, so this is ~8us per 2.1MB quad, ~64us
    total -- the DVE is the kernel's critical resource.  (DMA-routed
    alternatives through DRAM measured strictly worse: 512B-granule
    scatter reads OR writes collapse SDMA efficiency.)
  - stage 2: per n1-pair (32h+q16, 32h+q16+16): 2-way column-packed K=128
    matmuls (tile_position cols (0,0)/(0,64), same bank is fine for
    column packing), j=0/1 accumulate in 1-bank PSUM groups of 2 n1;
    drained with cast to fp16 and DMA'd out.  Pairing within an h-half
    lets stage-2 consume t2 per-h, so 4 rotating quad buffers suffice.
  - batch chunked (BC=256) x 2; emission interleaves st2(c-1) h-blocks
    into st1(c) so ACT drains, DVE transposes, and PE matmuls of
    adjacent chunks overlap.  DVE st1-drain shares live only in the j0
    half (j1 shares would queue behind transposes in the DVE FIFO and
    stall the ps1 rotation); stage-2 drains stay on ACT.
  - startup: the first x quarter and W1 piece go FIRST on the sync
    HWDGE ring so the SDMA packet round-robin cannot dilute them with
    later loads (all early loads on one FIFO ring in need-order; W2 on
    gpsimd after stage-1 emission begins).  Going finer than 0.5MB
    pieces measured worse (per-DMA issue+completion overhead).
  - tail: the last chunk's h1 stage-2 drains alternate ACT/DVE (both
    engines are idle then), collapsing the stage-2 MM->drain ping-pong;
    applying the same split mid-kernel measured worse (the DVE share
    queues behind transposes and delays them).  ysb bufs=6 decouples
    the drain chain from y-store WAR.
  - the early emission of st2(c-1) h-blocks inside st1(c) is
    load-bearing: it frees the rotating t2 quad buffers before chunk
    c's transposes need them.  Deferring st2(c-1) to chain the drain
    phases tighter measured 139us (transposes block on the quad WAR;
    no SBUF room for more quad buffers).

Self-contained: hardcodes all shapes; imports the Bass toolchain from
/opt/trn_rl_repo.
"""

import os
import re
import sys

import numpy as np

sys.path.insert(0, "/opt/trn_rl_repo")

import bass_rust  # noqa: E402
import concourse.bass as bass  # noqa: E402
import concourse.mybir as mybir  # noqa: E402
import concourse.tile as tile  # noqa: E402
from concourse import bass_utils  # noqa: E402

# ----------------------------------------------------------------------------
# Environment shims (same as baseline)
# ----------------------------------------------------------------------------


def _install_walrus_single_wait_patch():
    """This container's walrus build supports only ONE sem-wait per
    instruction. TileContext attaches several. Split every multi-wait
    instruction: hoist all-but-one wait onto same-engine NoOps placed
    immediately before it, and emit the tail drain one proc at a time."""
    if getattr(tile.TileContext, "_single_wait_patched", False):
        return

    counter = [0]

    def _split_multiwait_insts(ordered):
        for insts in ordered.values():
            i = 0
            while i < len(insts):
                inst = insts[i]
                si = getattr(inst, "sync_info", None)
                if si is not None and len(si.on_wait) > 1:
                    waits = list(si.on_wait)
                    new_nops = []
                    for w in waits[:-1]:
                        counter[0] += 1
                        nop = mybir.InstNoOp(
                            name=f"waitsplit_{counter[0]}", ins=[], outs=[]
                        )
                        nop.engine = inst.engine
                        nop.sync_info = bass_rust.SyncInfo(on_wait=[w], on_update=[])
                        new_nops.append(nop)
                    inst.sync_info = bass_rust.SyncInfo(
                        on_wait=[waits[-1]], on_update=list(si.on_update)
                    )
                    insts[i:i] = new_nops
                    i += len(new_nops)
                i += 1

    orig_lower = tile.TileContext._lower_ordered_insts

    def patched_lower(self, ordered):
        _split_multiwait_insts(ordered)
        return orig_lower(self, ordered)

    def split_drain_and_barrier(self, tick_clock, wait_clock):
        gc = tick_clock.global_clock
        ticks = [int(x) for x in re.findall(r"\d+", repr(gc.copy()))]
        emitted = False
        for i, t in enumerate(ticks):
            if t > 0:
                vec = [0] * len(ticks)
                vec[i] = t
                drain_inst = self.nc.sync.drain()
                wait_clock.add_sem_waits(
                    drain_inst.ins,
                    bass_rust.ScopedClock({None: bass_rust.VectorClock(vec)}),
                )
                emitted = True
        if not emitted:
            self.nc.sync.drain()
        self.nc.all_engine_barrier()
        assert self.sems is not None
        popped = self.nc._tile_sem_poison_stack.pop()
        assert popped is self._sem_poison
        self.nc.clear_and_free_semaphores(list(self.sems.allocated().values()))
        self.nc.all_engine_barrier()

    tile.TileContext._lower_ordered_insts = patched_lower
    tile.TileContext._drain_and_barrier = split_drain_and_barrier
    tile.TileContext._single_wait_patched = True


def _install_ntff_hook():
    """Register the NTFF profiling hook (missing antenv.axon_hooks module in
    this image). Only needed when profiling; harmless otherwise."""
    import types

    if "antenv.axon_hooks" not in sys.modules:
        import antenv

        mod = types.ModuleType("antenv.axon_hooks")
        mod._hook = None
        mod.set_axon_ntff_profile_hook = lambda h: setattr(mod, "_hook", h)
        mod.get_axon_ntff_profile_hook = lambda: mod._hook
        sys.modules["antenv.axon_hooks"] = mod
        antenv.axon_hooks = mod
    m = sys.modules["antenv.axon_hooks"]
    if m._hook is None:
        try:
            from trn_agent_boot.trn_boot import _ntff_profile_via_ctypes

            m.set_axon_ntff_profile_hook(
                _ntff_profile_via_ctypes("/opt/axon/libaxon_pjrt.so")
            )
        except Exception:
            pass
    bass_utils.upload_artifacts = lambda d: d


_install_walrus_single_wait_patch()

# ----------------------------------------------------------------------------
# Problem constants / tunables
# ----------------------------------------------------------------------------

B = 4096
M1 = M2 = N1 = N2 = 64
R = 4
NCORES = 8
BP = B // NCORES  # batch rows per core (512)

BC = int(os.environ.get("BTT_BC", "256"))  # batch chunk
# every Nth stage-1/stage-2 drain goes to DVE instead of ACT (0 = all ACT,
# 1 = all DVE)
S1_DVE_MOD = int(os.environ.get("BTT_S1_DVE_MOD", "10"))
S2_DVE_MOD = int(os.environ.get("BTT_S2_DVE_MOD", "1"))
# how many of the 4 per-chunk transpose quads go via DMA DRAM round-trip
# instead of the DVE stream-transpose (0..4, j0 quads first)
DMA_QUADS = int(os.environ.get("BTT_DMA_QUADS", "2"))


# ----------------------------------------------------------------------------
# Bass program
# ----------------------------------------------------------------------------


def build_program(bc=None, s1_dve_mod=None, s2_dve_mod=None, dma_quads=None):
    bc = bc or BC
    s1m = S1_DVE_MOD if s1_dve_mod is None else s1_dve_mod
    s2m = S2_DVE_MOD if s2_dve_mod is None else s2_dve_mod
    dmaq = DMA_QUADS if dma_quads is None else dma_quads
    nch = BP // bc
    f16 = mybir.dt.float16
    f32 = mybir.dt.float32
    sub = bc // 2  # matmul moving-column sub-chunk

    nc = bass.Bass(
        "TRN2",
        target_bir_lowering=False,
        debug=False,
        detect_race_conditions=os.environ.get("BTT_NO_RACE", "0") != "1",
    )

    # Host-marshalled layouts (see _marshal_inputs):
    #   xt[p][c][g][b] = x[core*BP + c*bc + b, (2g + p//64)*64 + p%64]
    #       (p = m1 + 64*(m2%2), g = m2//2)
    #   w1[p][g][c2]   = W1[2g + p//64, p%64, n1*4+r] with c2 = h*128+r*32+n1p,
    #       n1 = 32h + n1p
    #   w2[k][n1][j][n2] = W2[n1, (32j + k%32)*4 + k//32, n2]   (k = 32r + m2p)
    #   yt[p][c][qy][b] = y[core*BP + c*bc + b, n1*64 + n2]
    #       (n1 = 32*(qy//16) + qy%16 + 16*(p//64), n2 = p%64)
    xt_d = nc.dram_tensor("xt", [128, nch, 32, bc], f16, kind="ExternalInput")
    w1_d = nc.dram_tensor("w1", [128, 32, 256], f16, kind="ExternalInput")
    w2_d = nc.dram_tensor("w2", [128, 64, 2, 64], f16, kind="ExternalInput")
    yt_d = nc.dram_tensor("yt", [128, nch, 32, bc], f16, kind="ExternalOutput")

    with tile.TileContext(nc) as tc:
        with (
            tc.tile_pool(name="weights", bufs=1) as wpool,
            tc.tile_pool(name="xin", bufs=2) as xpool,
            tc.tile_pool(name="t1", bufs=1) as t1pool,
            tc.tile_pool(name="t2", bufs=4) as t2pool,
            tc.tile_pool(name="yout", bufs=6) as ypool,
            tc.tile_pool(name="ps1", bufs=3, space="PSUM") as ps1pool,
            tc.tile_pool(name="ps2", bufs=2, space="PSUM") as ps2pool,
            tc.tile_pool(name="dram", bufs=1, space="DRAM") as dram_pool,
        ):
            w1_sb = wpool.tile([128, 32, 256], f16, name="w1_sb")
            # DRAM scratch for DMA-routed transpose quads, slot per (c%2, j, h)
            qd_d = dram_pool.tile([2, 2, 2, 128, 32, bc], f16, name="qscratch")
            w2_sb = wpool.tile([128, 64, 2, 64], f16, name="w2_sb")

            # t1[j]: [128 p=32r+n1p][h][m2p][b]  (stage-1 output layout)
            t1_sb = [
                t1pool.tile([128, 2, 32, bc], f16, name=f"t1_{j}") for j in range(2)
            ]
            # t2 quads: [128 k=32r+m2p][n1p][b], rotated via tag (bufs=4)
            t2_sb = {}

            xg = [None] * nch

            def load_x(c, half=None, quarter=None):
                if quarter == 0 or half == 0 or (half is None and quarter is None):
                    xg[c] = xpool.tile([128, 32, bc], f16, tag="xg", name=f"xg_{c}")
                if half is None and quarter is None:
                    nc.sync.dma_start(xg[c][:], xt_d[:, c, :, :])
                elif quarter is not None:
                    nc.sync.dma_start(
                        xg[c][:, 8 * quarter : 8 * quarter + 8, :],
                        xt_d[:, c, 8 * quarter : 8 * quarter + 8, :],
                    )
                else:
                    nc.sync.dma_start(
                        xg[c][:, 16 * half : 16 * half + 16, :],
                        xt_d[:, c, 16 * half : 16 * half + 16, :],
                    )

            def load_w1(k, eng=None):
                (eng or nc.scalar).dma_start(
                    w1_sb[:, 8 * k : 8 * k + 8, :], w1_d[:, 8 * k : 8 * k + 8, :]
                )

            def load_w2():
                nc.gpsimd.dma_start(w2_sb[:, 0:32, :, :], w2_d[:, 0:32, :, :])
                nc.gpsimd.dma_start(w2_sb[:, 32:64, :, :], w2_d[:, 32:64, :, :])

            def stage1_half(c, jhalf):
                # g indexes an m2-pair (2g, 2g+1); j = m2//32 = g//16
                for g in range(16 * jhalf, 16 * jhalf + 16):
                    j = jhalf
                    mp = (2 * g) % 32
                    # p (row-group) must select the PSUM bank: concurrent
                    # row-packed matmuls cannot share a bank
                    ps = ps1pool.tile([128, 2, 2, sub * 2], f32, tag="ps1", name=f"ps1_{c}_{g}")
                    for h in range(2):
                        for p in range(2):
                            for s in range(2):
                                nc.tensor.matmul(
                                    ps[:, p, h, s * sub : (s + 1) * sub],
                                    w1_sb[
                                        64 * p : 64 * p + 64,
                                        g,
                                        128 * h : 128 * h + 128,
                                    ],
                                    xg[c][
                                        64 * p : 64 * p + 64,
                                        g,
                                        s * sub : (s + 1) * sub,
                                    ],
                                    start=True,
                                    stop=True,
                                )
                    # one (128, 1024) drain per g: PSUM fp32 -> t1 fp16
                    dst = t1_sb[j][:, :, mp : mp + 2, :]
                    src = ps.rearrange("c p h b -> c h p b")
                    if s1m > 0 and g % s1m == s1m - 1:
                        nc.vector.tensor_copy(dst, src)
                    else:
                        nc.scalar.copy(dst, src)
                    if c + 1 < nch and g % 16 == 1:
                        load_x(c + 1, jhalf)
                    if c == 0 and jhalf == 0 and g in (1, 3, 5):
                        load_w1((g + 1) // 2, eng=nc.sync)
                    if c == 0 and jhalf == 0 and g in (1, 2, 3):
                        load_x(0, quarter=g)

            def transpose(j, h, c):
                #   t2q[32r+m2p][n1p][b] = t1[j][32r+n1p][h][m2p][b]
                t2q = t2pool.tile([128, 32, bc], f16, tag="t2q", name=f"t2q_{j}_{h}")
                t2_sb[(j, h)] = t2q
                n_dma = [(0, 0), (0, 1), (1, 0), (1, 1)][:dmaq]
                if (j, h) in n_dma:
                    # DMA round-trip through DRAM with the shuffle applied on
                    # the WRITE side (posted writes tolerate 512B runs), then
                    # a fast contiguous read-back.  qd is in t2 layout
                    # [k=32r+m2p][n1p][b].
                    qd = qd_d[c % 2, j, h]
                    for r in range(R):
                        nc.scalar.dma_start(
                            qd[32 * r : 32 * r + 32].rearrange("m n b -> n m b"),
                            t1_sb[j][32 * r : 32 * r + 32, h],
                        )
                    nc.gpsimd.dma_start(t2q[:], qd[:])
                else:
                    # DVE 32x32 stream-transpose (fp32 pairs).  Mid-kernel
                    # quads run as one slice (less per-slice overhead); the
                    # final chunk's quads stay b-split so tail stage-2 can
                    # start on the first half.
                    nv = 2 if c == nch - 1 else 1
                    b2 = bc // 2 // nv  # fp32 units per slice
                    for v in range(nv):
                        in_ = (
                            t1_sb[j][:, h]
                            .bitcast(f32)[:, :, v * b2 : (v + 1) * b2]
                            .rearrange("p m b -> p b m")
                        )
                        out = (
                            t2q.bitcast(f32)[:, :, v * b2 : (v + 1) * b2]
                            .rearrange("p n b -> p b n")
                        )
                        nc.vector.transpose(out, in_)

            def stage2_half(c, h):
                # n1-pair (32h + q16, 32h + q16 + 16); consumes quads (*, h)
                for qg in range(8):
                    ps = ps2pool.tile([128, 2, bc], f32, tag="ps2", name=f"ps2_{c}_{h}_{qg}")
                    hb = bc // 2
                    for u in range(2):
                        q16 = 2 * qg + u
                        for v in range(2):
                            for j in range(2):
                                for pp in range(2):
                                    nc.tensor.matmul(
                                        ps[
                                            64 * pp : 64 * pp + 64,
                                            u,
                                            v * hb : (v + 1) * hb,
                                        ],
                                        w2_sb[:, 32 * h + q16 + 16 * pp, j, :],
                                        t2_sb[(j, h)][
                                            :, q16 + 16 * pp, v * hb : (v + 1) * hb
                                        ],
                                        start=(j == 0),
                                        stop=(j == 1),
                                    )
                    ysb = ypool.tile([128, 2, bc], f16, tag="ysb", name=f"ysb_{c}_{h}_{qg}")
                    use_dve = (s2m > 0 and qg % s2m == s2m - 1) or (
                        c == nch - 1 and h == 1 and qg % 2 == 1
                    )
                    if use_dve:
                        nc.vector.tensor_copy(ysb[:], ps[:])
                    else:
                        nc.scalar.copy(ysb[:], ps[:])
                    nc.sync.dma_start(
                        yt_d[:, c, 16 * h + 2 * qg : 16 * h + 2 * qg + 2, :], ysb[:]
                    )

            # ---- pipelined emission across chunks -------------------------
            load_x(0, quarter=0)
            load_w1(0, eng=nc.sync)
            stage1_half(0, 0)
            load_w2()
            transpose(0, 0, 0)
            transpose(0, 1, 0)
            stage1_half(0, 1)
            transpose(1, 0, 0)
            transpose(1, 1, 0)
            for c in range(1, nch):
                stage2_half(c - 1, 0)
                stage1_half(c, 0)
                stage2_half(c - 1, 1)
                transpose(0, 0, c)
                transpose(0, 1, c)
                stage1_half(c, 1)
                transpose(1, 0, c)
                transpose(1, 1, c)
            stage2_half(nch - 1, 0)
            stage2_half(nch - 1, 1)

    return nc


# ----------------------------------------------------------------------------
# Host marshalling
# ----------------------------------------------------------------------------


def _marshal_inputs(x, W1, W2, bc):
    nch = BP // bc
    # x: (B, 4096) -> xt_all (128, 32, B) with p = m1 + 64*(m2%2), g = m2//2
    xr = x.reshape(B, 32, 2, 64)  # [b][g][par][m1]
    xt_all = np.ascontiguousarray(
        xr.transpose(2, 3, 1, 0).reshape(128, 32, B).astype(np.float16)
    )
    # W1 (64 m2, 64 m1, 256 c=n1*4+r) -> w1[p][g][c2], c2 = h*128 + r*32 + n1p
    w1r = W1.reshape(32, 2, 64, 2, 32, 4)  # [g][par][m1][h][n1p][r]
    w1 = np.ascontiguousarray(
        w1r.transpose(1, 2, 0, 3, 5, 4).reshape(128, 32, 256).astype(np.float16)
    )
    # W2 (64 n1, 256 d=(32j+m2p)*4+r, 64 n2) -> w2[k=32r+m2p][n1][j][n2]
    w2r = W2.reshape(64, 2, 32, 4, 64)  # [n1][j][m2p][r][n2]
    w2 = np.ascontiguousarray(
        w2r.transpose(3, 2, 0, 1, 4).reshape(128, 64, 2, 64).astype(np.float16)
    )

    in_maps = []
    for core in range(NCORES):
        xc = xt_all[:, :, core * BP : (core + 1) * BP]  # (128, 32, BP)
        xc = np.ascontiguousarray(
            xc.reshape(128, 32, nch, bc).transpose(0, 2, 1, 3)
        )  # [p][c][g][b]
        in_maps.append({"xt": xc, "w1": w1, "w2": w2})
    return in_maps


def _unmarshal_output(results, bc):
    nch = BP // bc
    y = np.empty((B, N1 * N2), np.float32)
    for core, res in enumerate(results):
        yt = res["yt"]  # (128, nch, 32, bc) fp16
        # p = pp*64 + n2; qy = h*16 + q16; n1 = 32h + q16 + 16pp
        yc = (
            yt.reshape(2, 64, nch, 2, 16, bc)  # [pp][n2][c][h][q16][b]
            .transpose(2, 5, 3, 0, 4, 1)  # [c][b][h][pp][q16][n2]
            .reshape(BP, 4096)
            .astype(np.float32)
        )
        y[core * BP : (core + 1) * BP] = yc
    return y


# ----------------------------------------------------------------------------
# Public entry point
# ----------------------------------------------------------------------------

_PROGRAM_CACHE = {}


def kernel(x, W1, W2, _trace=False, _config=None):
    cfg = _config or {}
    key = tuple(sorted(cfg.items())) if cfg else None
    if key not in _PROGRAM_CACHE:
        _PROGRAM_CACHE[key] = build_program(**cfg)
    nc = _PROGRAM_CACHE[key]

    bc = cfg.get("bc", BC)
    in_maps = _marshal_inputs(
        np.asarray(x, np.float32),
        np.asarray(W1, np.float32),
        np.asarray(W2, np.float32),
        bc,
    )
    if _trace:
        _install_ntff_hook()
        os.environ["BASS_PERFETTO_PROFILE_ALL_CORES"] = "1"
    res = bass_utils.run_bass_kernel_spmd(
        nc, in_maps, core_ids=list(range(NCORES)), trace=_trace
    )
    y = _unmarshal_output(res.results, bc)
    if _trace:
        return y, res
    return y


# revision 33
# speedup vs baseline: 1.2309x; 1.0488x over previous
"""BTT layer kernel for Trainium2 (8 NeuronCores, data-parallel over batch).

Computes y = BTT(x; W1, W2) where
  x: (4096, 4096) fp32, W1: (64, 64, 256) fp32, W2: (64, 256, 64) fp32
  stage 1: t[b, m2, n1, r] = sum_m1 x[b, m2, m1] * W1[m2, m1, n1*4+r]
  stage 2: y[b, n1, n2]   = sum_{m2, r} t[b, m2, n1, r] * W2[n1, m2*4+r, n2]

Design (SBUF-resident intermediate, no DRAM round-trip; ~127us HW,
rel err 5.5e-4; the previous DRAM-round-trip version measured ~190us):
  - everything fp16 on device (x, W1, W2, t, y); host casts / uncasts.
  - stage 1: per m2-pair: 2-way row-packed K=64 matmuls (tile_position rows
    (0,0)/(64,0)) fill a 2-bank PSUM tile (concurrent row-group matmuls
    must land in DIFFERENT PSUM banks); ACT/DVE copies (cast to fp16)
    drain into t1[j] tiles laid out [128 p=32r+n1p][h][m2p][b].  Drains
    are the throughput wall: fp32 PSUM sources run at 1 elem/cycle/lane
    on both DVE (0.96 GHz) and ACT (1.2 GHz); GPSIMD has no PSUM port.
  - the BTT "transpose" (n1p <-> m2p inside 32-lane banks; r rides as the
    bank index) runs on the DVE stream-transpose as fp32 *pairs* of
    b-adjacent fp16 values, landing t2 quads [128 k=32r+m2p][n1p][b]
    directly in stage-2 moving-operand layout.  The reshape front-end
    moves 16 bits/lane/cycle, so this is ~8us per 2.1MB quad, ~64us
    total -- the DVE is the kernel's critical resource.  (DMA-routed
    alternatives through DRAM measured strictly worse: 512B-granule
    scatter reads OR writes collapse SDMA efficiency.)
  - stage 2: per n1-pair (32h+q16, 32h+q16+16): 2-way column-packed K=128
    matmuls (tile_position cols (0,0)/(0,64), same bank is fine for
    column packing), j=0/1 accumulate in 1-bank PSUM groups of 2 n1;
    drained with cast to fp16 and DMA'd out.  Pairing within an h-half
    lets stage-2 consume t2 per-h, so 4 rotating quad buffers suffice.
  - batch chunked (BC=256) x 2; emission interleaves st2(c-1) h-blocks
    into st1(c) so ACT drains, DVE transposes, and PE matmuls of
    adjacent chunks overlap.  DVE st1-drain shares live only in the j0
    half (j1 shares would queue behind transposes in the DVE FIFO and
    stall the ps1 rotation); stage-2 drains stay on ACT.
  - startup: the first x quarter and W1 piece go FIRST on the sync
    HWDGE ring so the SDMA packet round-robin cannot dilute them with
    later loads (all early loads on one FIFO ring in need-order; W2 on
    gpsimd after stage-1 emission begins).  Going finer than 0.5MB
    pieces measured worse (per-DMA issue+completion overhead).
  - tail: the last chunk's h1 stage-2 drains alternate ACT/DVE (both
    engines are idle then), collapsing the stage-2 MM->drain ping-pong;
    applying the same split mid-kernel measured worse (the DVE share
    queues behind transposes and delays them).  ysb bufs=6 decouples
    the drain chain from y-store WAR.
  - the early emission of st2(c-1) h-blocks inside st1(c) is
    load-bearing: it frees the rotating t2 quad buffers before chunk
    c's transposes need them.  Deferring st2(c-1) to chain the drain
    phases tighter measured 139us (transposes block on the quad WAR;
    no SBUF room for more quad buffers).

Self-contained: hardcodes all shapes; imports the Bass toolchain from
/opt/trn_rl_repo.
"""

import os
import re
import sys

import numpy as np

sys.path.insert(0, "/opt/trn_rl_repo")

import bass_rust  # noqa: E402
import concourse.bass as bass  # noqa: E402
import concourse.mybir as mybir  # noqa: E402
import concourse.tile as tile  # noqa: E402
from concourse import bass_utils  # noqa: E402

# ----------------------------------------------------------------------------
# Environment shims (same as baseline)
# ----------------------------------------------------------------------------


def _install_walrus_single_wait_patch():
    """This container's walrus build supports only ONE sem-wait per
    instruction. TileContext attaches several. Split every multi-wait
    instruction: hoist all-but-one wait onto same-engine NoOps placed
    immediately before it, and emit the tail drain one proc at a time."""
    if getattr(tile.TileContext, "_single_wait_patched", False):
        return

    counter = [0]

    def _split_multiwait_insts(ordered):
        for insts in ordered.values():
            i = 0
            while i < len(insts):
                inst = insts[i]
                si = getattr(inst, "sync_info", None)
                if si is not None and len(si.on_wait) > 1:
                    waits = list(si.on_wait)
                    new_nops = []
                    for w in waits[:-1]:
                        counter[0] += 1
                        nop = mybir.InstNoOp(
                            name=f"waitsplit_{counter[0]}", ins=[], outs=[]
                        )
                        nop.engine = inst.engine
                        nop.sync_info = bass_rust.SyncInfo(on_wait=[w], on_update=[])
                        new_nops.append(nop)
                    inst.sync_info = bass_rust.SyncInfo(
                        on_wait=[waits[-1]], on_update=list(si.on_update)
                    )
                    insts[i:i] = new_nops
                    i += len(new_nops)
                i += 1

    orig_lower = tile.TileContext._lower_ordered_insts

    def patched_lower(self, ordered):
        _split_multiwait_insts(ordered)
        return orig_lower(self, ordered)

    def split_drain_and_barrier(self, tick_clock, wait_clock):
        gc = tick_clock.global_clock
        ticks = [int(x) for x in re.findall(r"\d+", repr(gc.copy()))]
        emitted = False
        for i, t in enumerate(ticks):
            if t > 0:
                vec = [0] * len(ticks)
                vec[i] = t
                drain_inst = self.nc.sync.drain()
                wait_clock.add_sem_waits(
                    drain_inst.ins,
                    bass_rust.ScopedClock({None: bass_rust.VectorClock(vec)}),
                )
                emitted = True
        if not emitted:
            self.nc.sync.drain()
        self.nc.all_engine_barrier()
        assert self.sems is not None
        popped = self.nc._tile_sem_poison_stack.pop()
        assert popped is self._sem_poison
        self.nc.clear_and_free_semaphores(list(self.sems.allocated().values()))
        self.nc.all_engine_barrier()

    tile.TileContext._lower_ordered_insts = patched_lower
    tile.TileContext._drain_and_barrier = split_drain_and_barrier
    tile.TileContext._single_wait_patched = True


def _install_ntff_hook():
    """Register the NTFF profiling hook (missing antenv.axon_hooks module in
    this image). Only needed when profiling; harmless otherwise."""
    import types

    if "antenv.axon_hooks" not in sys.modules:
        import antenv

        mod = types.ModuleType("antenv.axon_hooks")
        mod._hook = None
        mod.set_axon_ntff_profile_hook = lambda h: setattr(mod, "_hook", h)
        mod.get_axon_ntff_profile_hook = lambda: mod._hook
        sys.modules["antenv.axon_hooks"] = mod
        antenv.axon_hooks = mod
    m = sys.modules["antenv.axon_hooks"]
    if m._hook is None:
        try:
            from trn_agent_boot.trn_boot import _ntff_profile_via_ctypes

            m.set_axon_ntff_profile_hook(
                _ntff_profile_via_ctypes("/opt/axon/libaxon_pjrt.so")
            )
        except Exception:
            pass
    bass_utils.upload_artifacts = lambda d: d


_install_walrus_single_wait_patch()

# ----------------------------------------------------------------------------
# Problem constants / tunables
# ----------------------------------------------------------------------------

B = 4096
M1 = M2 = N1 = N2 = 64
R = 4
NCORES = 8
BP = B // NCORES  # batch rows per core (512)

BC = int(os.environ.get("BTT_BC", "256"))  # batch chunk
# every Nth stage-1/stage-2 drain goes to DVE instead of ACT (0 = all ACT,
# 1 = all DVE)
S1_DVE_MOD = int(os.environ.get("BTT_S1_DVE_MOD", "10"))
S2_DVE_MOD = int(os.environ.get("BTT_S2_DVE_MOD", "1"))
# how many of the 4 per-chunk transpose quads go via DMA DRAM round-trip
# instead of the DVE stream-transpose (0..4, j0 quads first)
DMA_QUADS = int(os.environ.get("BTT_DMA_QUADS", "2"))


# ----------------------------------------------------------------------------
# Bass program
# ----------------------------------------------------------------------------


def build_program(bc=None, s1_dve_mod=None, s2_dve_mod=None, dma_quads=None):
    bc = bc or BC
    s1m = S1_DVE_MOD if s1_dve_mod is None else s1_dve_mod
    s2m = S2_DVE_MOD if s2_dve_mod is None else s2_dve_mod
    dmaq = DMA_QUADS if dma_quads is None else dma_quads
    nch = BP // bc
    f16 = mybir.dt.float16
    f32 = mybir.dt.float32
    sub = bc // 2  # matmul moving-column sub-chunk

    nc = bass.Bass(
        "TRN2",
        target_bir_lowering=False,
        debug=False,
        detect_race_conditions=os.environ.get("BTT_NO_RACE", "0") != "1",
    )

    # Host-marshalled layouts (see _marshal_inputs):
    #   xt[p][c][g][b] = x[core*BP + c*bc + b, (2g + p//64)*64 + p%64]
    #       (p = m1 + 64*(m2%2), g = m2//2)
    #   w1[p][g][c2]   = W1[2g + p//64, p%64, n1*4+r] with c2 = h*128+r*32+n1p,
    #       n1 = 32h + n1p
    #   w2[k][n1][j][n2] = W2[n1, (32j + k%32)*4 + k//32, n2]   (k = 32r + m2p)
    #   yt[p][c][qy][b] = y[core*BP + c*bc + b, n1*64 + n2]
    #       (n1 = 32*(qy//16) + qy%16 + 16*(p//64), n2 = p%64)
    xt_d = nc.dram_tensor("xt", [128, nch, 32, bc], f16, kind="ExternalInput")
    w1_d = nc.dram_tensor("w1", [128, 32, 256], f16, kind="ExternalInput")
    w2_d = nc.dram_tensor("w2", [128, 64, 2, 64], f16, kind="ExternalInput")
    yt_d = nc.dram_tensor("yt", [128, nch, 32, bc], f16, kind="ExternalOutput")

    with tile.TileContext(nc) as tc:
        with (
            tc.tile_pool(name="weights", bufs=1) as wpool,
            tc.tile_pool(name="xin", bufs=2) as xpool,
            tc.tile_pool(name="t1", bufs=1) as t1pool,
            tc.tile_pool(name="t2", bufs=5) as t2pool,
            tc.tile_pool(name="yout", bufs=4) as ypool,
            tc.tile_pool(name="ps1", bufs=3, space="PSUM") as ps1pool,
            tc.tile_pool(name="ps2", bufs=2, space="PSUM") as ps2pool,
            tc.tile_pool(name="dram", bufs=1, space="DRAM") as dram_pool,
        ):
            w1_sb = wpool.tile([128, 32, 256], f16, name="w1_sb")
            # DRAM scratch for DMA-routed transpose quads, slot per (c%2, j, h)
            qd_d = dram_pool.tile([2, 2, 2, 128, 32, bc], f16, name="qscratch")
            w2_sb = wpool.tile([128, 64, 2, 64], f16, name="w2_sb")

            # t1[j]: [128 p=32r+n1p][h][m2p][b]  (stage-1 output layout)
            t1_sb = [
                t1pool.tile([128, 2, 32, bc], f16, name=f"t1_{j}") for j in range(2)
            ]
            # t2 quads: [128 k=32r+m2p][n1p][b], rotated via tag (bufs=4)
            t2_sb = {}

            xg = {}

            def load_x(c, half, quarter=None):
                key = (c, half)
                if key not in xg:
                    xg[key] = xpool.tile(
                        [128, 16, bc], f16, tag="xgh", name=f"xg_{c}_{half}"
                    )
                if quarter is None:
                    nc.sync.dma_start(
                        xg[key][:], xt_d[:, c, 16 * half : 16 * half + 16, :]
                    )
                else:
                    nc.sync.dma_start(
                        xg[key][:, 8 * quarter : 8 * quarter + 8, :],
                        xt_d[
                            :,
                            c,
                            16 * half + 8 * quarter : 16 * half + 8 * quarter + 8,
                            :,
                        ],
                    )

            def load_w1(k, eng=None):
                (eng or nc.scalar).dma_start(
                    w1_sb[:, 8 * k : 8 * k + 8, :], w1_d[:, 8 * k : 8 * k + 8, :]
                )

            def load_w2():
                nc.gpsimd.dma_start(w2_sb[:, 0:32, :, :], w2_d[:, 0:32, :, :])
                nc.gpsimd.dma_start(w2_sb[:, 32:64, :, :], w2_d[:, 32:64, :, :])

            def stage1_half(c, jhalf):
                # g indexes an m2-pair (2g, 2g+1); j = m2//32 = g//16
                for g in range(16 * jhalf, 16 * jhalf + 16):
                    j = jhalf
                    mp = (2 * g) % 32
                    # p (row-group) must select the PSUM bank: concurrent
                    # row-packed matmuls cannot share a bank
                    ps = ps1pool.tile([128, 2, 2, sub * 2], f32, tag="ps1", name=f"ps1_{c}_{g}")
                    for h in range(2):
                        for p in range(2):
                            for s in range(2):
                                nc.tensor.matmul(
                                    ps[:, p, h, s * sub : (s + 1) * sub],
                                    w1_sb[
                                        64 * p : 64 * p + 64,
                                        g,
                                        128 * h : 128 * h + 128,
                                    ],
                                    xg[(c, g // 16)][
                                        64 * p : 64 * p + 64,
                                        g % 16,
                                        s * sub : (s + 1) * sub,
                                    ],
                                    start=True,
                                    stop=True,
                                )
                    # one (128, 1024) drain per g: PSUM fp32 -> t1 fp16
                    dst = t1_sb[j][:, :, mp : mp + 2, :]
                    src = ps.rearrange("c p h b -> c h p b")
                    if s1m > 0 and g % s1m == s1m - 1:
                        nc.vector.tensor_copy(dst, src)
                    else:
                        nc.scalar.copy(dst, src)
                    if c + 1 < nch and g % 16 == 7:
                        load_x(c + 1, jhalf)
                    if c == 0 and jhalf == 0 and g in (1, 3, 5):
                        load_w1((g + 1) // 2, eng=nc.sync)
                    if c == 0 and jhalf == 0 and g == 2:
                        load_x(0, 0, quarter=1)
                    if c == 0 and jhalf == 0 and g == 4:
                        load_x(0, 1)

            def transpose(j, h, c):
                #   t2q[32r+m2p][n1p][b] = t1[j][32r+n1p][h][m2p][b]
                t2q = t2pool.tile([128, 32, bc], f16, tag="t2q", name=f"t2q_{j}_{h}")
                t2_sb[(j, h)] = t2q
                n_dma = [(0, 0), (0, 1), (1, 0), (1, 1)][:dmaq]
                if (j, h) in n_dma:
                    # DMA round-trip through DRAM with the shuffle applied on
                    # the WRITE side (posted writes tolerate 512B runs), then
                    # a fast contiguous read-back.  qd is in t2 layout
                    # [k=32r+m2p][n1p][b].
                    qd = qd_d[c % 2, j, h]
                    for r in range(R):
                        nc.scalar.dma_start(
                            qd[32 * r : 32 * r + 32].rearrange("m n b -> n m b"),
                            t1_sb[j][32 * r : 32 * r + 32, h],
                        )
                    nc.gpsimd.dma_start(t2q[:], qd[:])
                else:
                    # DVE 32x32 stream-transpose (fp32 pairs).  Mid-kernel
                    # quads run as one slice (less per-slice overhead); the
                    # final chunk's quads stay b-split so tail stage-2 can
                    # start on the first half.
                    nv = 2 if c == nch - 1 else 1
                    b2 = bc // 2 // nv  # fp32 units per slice
                    for v in range(nv):
                        in_ = (
                            t1_sb[j][:, h]
                            .bitcast(f32)[:, :, v * b2 : (v + 1) * b2]
                            .rearrange("p m b -> p b m")
                        )
                        out = (
                            t2q.bitcast(f32)[:, :, v * b2 : (v + 1) * b2]
                            .rearrange("p n b -> p b n")
                        )
                        nc.vector.transpose(out, in_)

            def stage2_half(c, h):
                # n1-pair (32h + q16, 32h + q16 + 16); consumes quads (*, h)
                for qg in range(8):
                    ps = ps2pool.tile([128, 2, bc], f32, tag="ps2", name=f"ps2_{c}_{h}_{qg}")
                    hb = bc // 2
                    for u in range(2):
                        q16 = 2 * qg + u
                        for v in range(2):
                            for j in range(2):
                                for pp in range(2):
                                    nc.tensor.matmul(
                                        ps[
                                            64 * pp : 64 * pp + 64,
                                            u,
                                            v * hb : (v + 1) * hb,
                                        ],
                                        w2_sb[:, 32 * h + q16 + 16 * pp, j, :],
                                        t2_sb[(j, h)][
                                            :, q16 + 16 * pp, v * hb : (v + 1) * hb
                                        ],
                                        start=(j == 0),
                                        stop=(j == 1),
                                    )
                    ysb = ypool.tile([128, 2, bc], f16, tag="ysb", name=f"ysb_{c}_{h}_{qg}")
                    use_dve = (s2m > 0 and qg % s2m == s2m - 1) or (
                        c == nch - 1 and h == 1 and qg % 2 == 1
                    )
                    if use_dve:
                        nc.vector.tensor_copy(ysb[:], ps[:])
                    else:
                        nc.scalar.copy(ysb[:], ps[:])
                    nc.sync.dma_start(
                        yt_d[:, c, 16 * h + 2 * qg : 16 * h + 2 * qg + 2, :], ysb[:]
                    )

            # ---- pipelined emission across chunks -------------------------
            load_x(0, 0, quarter=0)
            load_w1(0, eng=nc.sync)
            stage1_half(0, 0)
            load_w2()
            transpose(0, 0, 0)
            transpose(0, 1, 0)
            stage1_half(0, 1)
            transpose(1, 0, 0)
            transpose(1, 1, 0)
            for c in range(1, nch):
                stage1_half(c, 0)
                stage1_half(c, 1)
                stage2_half(c - 1, 0)
                stage2_half(c - 1, 1)
                transpose(0, 0, c)
                transpose(0, 1, c)
                transpose(1, 0, c)
                transpose(1, 1, c)
            stage2_half(nch - 1, 0)
            stage2_half(nch - 1, 1)

    return nc


# ----------------------------------------------------------------------------
# Host marshalling
# ----------------------------------------------------------------------------


def _marshal_inputs(x, W1, W2, bc):
    nch = BP // bc
    # x: (B, 4096) -> xt_all (128, 32, B) with p = m1 + 64*(m2%2), g = m2//2
    xr = x.reshape(B, 32, 2, 64)  # [b][g][par][m1]
    xt_all = np.ascontiguousarray(
        xr.transpose(2, 3, 1, 0).reshape(128, 32, B).astype(np.float16)
    )
    # W1 (64 m2, 64 m1, 256 c=n1*4+r) -> w1[p][g][c2], c2 = h*128 + r*32 + n1p
    w1r = W1.reshape(32, 2, 64, 2, 32, 4)  # [g][par][m1][h][n1p][r]
    w1 = np.ascontiguousarray(
        w1r.transpose(1, 2, 0, 3, 5, 4).reshape(128, 32, 256).astype(np.float16)
    )
    # W2 (64 n1, 256 d=(32j+m2p)*4+r, 64 n2) -> w2[k=32r+m2p][n1][j][n2]
    w2r = W2.reshape(64, 2, 32, 4, 64)  # [n1][j][m2p][r][n2]
    w2 = np.ascontiguousarray(
        w2r.transpose(3, 2, 0, 1, 4).reshape(128, 64, 2, 64).astype(np.float16)
    )

    in_maps = []
    for core in range(NCORES):
        xc = xt_all[:, :, core * BP : (core + 1) * BP]  # (128, 32, BP)
        xc = np.ascontiguousarray(
            xc.reshape(128, 32, nch, bc).transpose(0, 2, 1, 3)
        )  # [p][c][g][b]
        in_maps.append({"xt": xc, "w1": w1, "w2": w2})
    return in_maps


def _unmarshal_output(results, bc):
    nch = BP // bc
    y = np.empty((B, N1 * N2), np.float32)
    for core, res in enumerate(results):
        yt = res["yt"]  # (128, nch, 32, bc) fp16
        # p = pp*64 + n2; qy = h*16 + q16; n1 = 32h + q16 + 16pp
        yc = (
            yt.reshape(2, 64, nch, 2, 16, bc)  # [pp][n2][c][h][q16][b]
            .transpose(2, 5, 3, 0, 4, 1)  # [c][b][h][pp][q16][n2]
            .reshape(BP, 4096)
            .astype(np.float32)
        )
        y[core * BP : (core + 1) * BP] = yc
    return y


# ----------------------------------------------------------------------------
# Public entry point
# ----------------------------------------------------------------------------

_PROGRAM_CACHE = {}


def kernel(x, W1, W2, _trace=False, _config=None):
    cfg = _config or {}
    key = tuple(sorted(cfg.items())) if cfg else None
    if key not in _PROGRAM_CACHE:
        _PROGRAM_CACHE[key] = build_program(**cfg)
    nc = _PROGRAM_CACHE[key]

    bc = cfg.get("bc", BC)
    in_maps = _marshal_inputs(
        np.asarray(x, np.float32),
        np.asarray(W1, np.float32),
        np.asarray(W2, np.float32),
        bc,
    )
    if _trace:
        _install_ntff_hook()
        os.environ["BASS_PERFETTO_PROFILE_ALL_CORES"] = "1"
    res = bass_utils.run_bass_kernel_spmd(
        nc, in_maps, core_ids=list(range(NCORES)), trace=_trace
    )
    y = _unmarshal_output(res.results, bc)
    if _trace:
        return y, res
    return y
